# revision 32
# baseline (speedup 1.0000x reference)
"""Causal self-attention with ALiBi for TRN2, 8 NeuronCores.

Sharding: core c -> batch b = c % 4, head-shard hs = c // 4.
Head-shard hs owns global heads {2j + hs : j in 0..7} (interleaved so both
shards see the same mix of ALiBi slopes -> balanced banded-attention work).

Key HW facts this kernel is built around (measured on TRN2):
  * fp32/f32r matmuls run in multi-pass fp32_mode=HIGH and disable fast
    weight load; bf16 matmuls issue at N/2.4GHz with ~100ns LDWEIGHTS
    overlapped.  All matmul operands here are bf16 (PSUM accum is fp32).
  * the HAM clock gate runs the PE at 1.2 GHz until ~3.4us of sustained
    activity; a warm-up matmul burst on the first-loaded const tile keeps
    the PE busy during the input DMA so real work starts at 2.4 GHz.
  * per-matmul LDWEIGHTS makes many small DMAs/matmuls expensive; weights
    are packed host-side into one DMA per pair (QK), one for V, one for Wo.
  * reciprocal_approx_fast CANNOT read PSUM directly (garbage, no error):
    the denominator row is staged through SBUF first.

Per-core computation (B=1 batch, 8 heads):
  V is produced in [s, col] layout with a ones column appended per head
  (the PV matmul then yields both the unnormalized output AND the softmax
  denominator).  Per pair p (slots 2p, 2p+1): Q^T/K^T in [col, s] layout
  (head pairs packed 64+64 into 128-partition tiles, Q pre-scaled by
  1/sqrt(HD) via host-scaled Wq), then per (head, q-chunk 512):
  scores S^T[k,q] = K^T.T @ Q^T on PE (K=64 contraction, head pairs at
  base partitions 0/64), exp on ACT with per-partition bias
  slope*(k - qmid) (the -slope*q part of ALiBi cancels in softmax; qmid
  recentering prevents overflow; band truncation at CUT=12 skips k-tiles
  whose weights vanish), PV accumulation into PSUM [65, 512] over the k
  band, then normalize (reciprocal of row 64, broadcast via gpsimd) into
  OT (bf16).  Pairs run heavy-to-light (3,2,1,0) so the ACT exp stream
  overlaps the dense PE projection stream; the out-projection
  O^T.T @ Wo runs at the end over all 16 s-tiles.

Host side: shard/transpose/bf16-convert/pack inputs, run SPMD on 8 cores,
sum the two head-shards' partial outputs per batch, add bo.
"""

import math

import numpy as np

B, S, D, H = 4, 2048, 1024, 16
HD = D // H
NSLOT = 8          # local heads per core
NQC = 4            # q chunks of 512
NKT = 16           # k tiles of 128
SC = 512
KT = 128
NCORES = 8

# ALiBi slopes for global heads
SLOPES = [2.0 ** (-0.5 * (h + 1)) for h in range(H)]

# band cutoff: terms with slope*(q-k) > CUT are < e^-CUT relative to the
# diagonal term and invisible next to the bf16 matmul noise (~4e-3)
CUT = 10.0


def _bt(h):
    """Band width in 128-k-tiles for global head h (delta_max + 1)."""
    d_max = int(math.ceil(CUT / SLOPES[h]))
    return min(NKT, (127 + d_max) // 128 + 1)


def _w(h):
    """Max exp-op width (q columns) for global head h: slope*(W/2) <= 64
    (bounds the exp dynamic range across a recentered column block)."""
    s = SLOPES[h]
    if s * 256.0 <= 64.0:
        return 512
    if s * 128.0 <= 64.0:
        return 256
    return 128


# per-slot params = union over the two head shards (program is SPMD-shared)
SLOT_BT = [max(_bt(2 * j), _bt(2 * j + 1)) for j in range(NSLOT)]
SLOT_W = [min(_w(2 * j), _w(2 * j + 1)) for j in range(NSLOT)]


def plan_attention():
    """Enumerate all attention tile ops. Returns (ops, bias_cols) where ops is
    a list of dicts and bias_cols maps (slot, mkey) -> expb column index."""
    bias_cols = {}
    ops = []
    for p in range(4):
        for qc in range(NQC):
            for kt in range(4 * qc + 4):
                for half in (0, 1):
                    j = 2 * p + half
                    bt, w = SLOT_BT[j], SLOT_W[j]
                    lo = max(0, 4 * qc - bt + 1)
                    if kt < lo:
                        continue
                    qs_start = max(4 * qc, kt)
                    qs_end = min(4 * qc + 3, kt + bt - 1)
                    if qs_start > qs_end:
                        continue
                    c0 = 128 * (qs_start - 4 * qc)
                    c1 = 128 * (qs_end - 4 * qc) + 128
                    # exp ops aligned to an absolute w-grid within the qc
                    # chunk: qmid (the recentering constant) must depend only
                    # on the column block, never on kt, so that every term
                    # entering a given column's softmax sum carries the same
                    # exp(-slope*qmid) factor.
                    exps = []
                    for g in range((c0 // w) * w, c1, w):
                        a, e = max(c0, g), min(c1, g + w)
                        if a >= e:
                            continue
                        mkey = (512 * qc + g + w // 2) - 128 * kt
                        col = bias_cols.setdefault((j, mkey), len(bias_cols))
                        exps.append((a, e - a, col))
                    ops.append(dict(qc=qc, p=p, half=half, j=j, kt=kt,
                                    c0=c0, c1=c1, exps=exps,
                                    tril=(kt >= 4 * qc),
                                    first=(kt == lo), last=(kt == 4 * qc + 3)))
    return ops, bias_cols


ATT_OPS, BIAS_COLS = plan_attention()
NBIAS = len(BIAS_COLS)

_nc_cache = None


def build_program():
    global _nc_cache
    if _nc_cache is not None:
        return _nc_cache

    import concourse.bacc as bacc
    import concourse.tile as tile
    from concourse import mybir

    F32 = mybir.dt.float32
    BF16 = mybir.dt.bfloat16
    EXP = mybir.ActivationFunctionType.Exp
    COPY = mybir.ActivationFunctionType.Copy

    nc = bacc.Bacc("TRN2", target_bir_lowering=False, debug=False,
                   num_devices=NCORES)

    xT_d = nc.dram_tensor("xT", [D, S], BF16, kind="ExternalInput")
    # packed weights: one DMA each.  wqk[p]: [128, d(8) x (Q128 | K128)];
    # wvp: [128, d(8) x 512 vcols]; wop: [128, f(4) x 1024 dcols]
    wqk_d = [nc.dram_tensor(f"wqk{p}", [128, 2048], BF16,
                            kind="ExternalInput") for p in range(4)]
    wvp_d = nc.dram_tensor("wvp", [128, 4096], BF16, kind="ExternalInput")
    wop_d = nc.dram_tensor("wop", [128, 4096], BF16, kind="ExternalInput")
    qkb_d = nc.dram_tensor("qkb", [128, 8], F32, kind="ExternalInput")
    bvr_d = nc.dram_tensor("bvr", [128, 512], F32, kind="ExternalInput")
    expb_d = nc.dram_tensor("expb", [128, max(NBIAS, 1)], F32,
                            kind="ExternalInput")
    tril_d = nc.dram_tensor("tril", [128, 128], BF16, kind="ExternalInput")
    vones_d = nc.dram_tensor("vones", [128, 8], BF16, kind="ExternalInput")
    out_d = nc.dram_tensor("out_p", [S, D], BF16, kind="ExternalOutput")

    ops_by_p = {p: [o for o in ATT_OPS if o["p"] == p] for p in range(4)}

    with tile.TileContext(nc) as tc:
        with nc.allow_low_precision(reason="bf16 attention kernel"), \
             tc.tile_pool(name="persist", bufs=1) as pp, \
             tc.tile_pool(name="expsp", bufs=8) as expsp, \
             tc.tile_pool(name="wqkp", bufs=2) as wqkp, \
             tc.tile_pool(name="rcp", bufs=4) as rcp, \
             tc.tile_pool(name="rbp", bufs=4) as rbp, \
             tc.tile_pool(name="outp", bufs=8) as outp, \
             tc.tile_pool(name="qkps", bufs=2, space="PSUM") as qkps, \
             tc.tile_pool(name="sps", bufs=3, space="PSUM") as sps, \
             tc.tile_pool(name="ops_", bufs=3, space="PSUM") as ops_:

            # ---- persistent tiles ----
            qkT_Q = [pp.tile([128, S], BF16, name=f"qkTQ{p}") for p in range(4)]
            # K stationaries, zero-padded to full 128-row contraction: zqk[p][h]
            # has half h's K dims in its own 64 partitions and ZEROS in the
            # other 64, so score matmuls use uniform full-array stationaries
            # (background weight-buffer overlap works; no partial row-groups).
            zqk = [[pp.tile([128, S], BF16, name=f"zqk{p}_{h}") for h in (0, 1)]
                   for p in range(4)]
            Vbuf = [pp.tile([128, NSLOT * 65], BF16, name=f"vb{t}")
                    for t in range(NKT)]
            OT = [pp.tile([128, S], BF16, name=f"OT{p}") for p in range(4)]
            xT = [pp.tile([128, S], BF16, name=f"xT{d}") for d in range(8)]
            wv = pp.tile([128, 4096], BF16, name="wv")
            wo_t = pp.tile([128, 4096], BF16, name="wo_t")
            qkb_t = pp.tile([128, 8], F32, name="qkb_t")
            bvr_t = pp.tile([128, 512], F32, name="bvr_t")
            expb_t = pp.tile([128, max(NBIAS, 1)], F32, name="expb_t")
            tril_t = pp.tile([128, 128], BF16, name="tril_t")
            vones_t = pp.tile([128, 8], BF16, name="vones_t")

            # ---- input DMA: tril first (feeds the PE warm-up), then xT
            # round-robin over the 3 DMA-capable queues; first pair's packed
            # weights lead the sync queue ----
            nc.gpsimd.dma_start(out=tril_t, in_=tril_d[:, :])
            nc.gpsimd.dma_start(out=qkb_t, in_=qkb_d[:, :])
            nc.gpsimd.dma_start(out=expb_t, in_=expb_d[:, :])
            nc.gpsimd.dma_start(out=vones_t, in_=vones_d[:, :])

            wqk_t = {}
            wqk_t[3] = wqkp.tile([128, 2048], BF16, name="wqk")
            nc.sync.dma_start(out=wqk_t[3], in_=wqk_d[3][:, :])
            for d in range(8):
                eng = (nc.gpsimd, nc.scalar, nc.sync)[d % 3]
                eng.dma_start(out=xT[d], in_=xT_d[128 * d:128 * (d + 1), :])
            nc.sync.dma_start(out=wv, in_=wvp_d[:, :])
            nc.gpsimd.dma_start(out=bvr_t, in_=bvr_d[:, :])
            nc.scalar.dma_start(out=wo_t, in_=wop_d[:, :])

            # zero the unused halves of the zqk stationaries (once)
            for p in range(4):
                eng = nc.vector if p % 2 == 0 else nc.gpsimd
                eng.memset(zqk[p][0][64:128, :], 0.0)
                eng.memset(zqk[p][1][0:64, :], 0.0)

            # ones columns of Vbuf (col 64 of each 65-wide head group)
            for t in range(NKT):
                ones_view = Vbuf[t].rearrange("p (h c) -> p h c", c=65)[:, :, 64:65]
                nc.vector.tensor_copy(ones_view, vones_t.unsqueeze(2))

            def proj_qk(p):
                """QK projection for pair p into qkT_Q[p]/qkT_K[p] (bf16)."""
                for m in (0, 1):       # 0 = Q cols, 1 = K cols
                    for sh in range(2):
                        psq = [qkps.tile([128, SC], F32, name="psq", tag="ps1")
                               for _ in range(2)]
                        for d in range(8):
                            for si in range(2):
                                s = 2 * sh + si
                                nc.tensor.matmul(
                                    psq[si],
                                    wqk_t[p][:, 256 * d + 128 * m:
                                             256 * d + 128 * (m + 1)],
                                    xT[d][:, SC * s:SC * (s + 1)],
                                    start=(d == 0), stop=(d == 7))
                        for si in range(2):
                            s = 2 * sh + si
                            if m == 0:
                                nc.vector.tensor_scalar_add(
                                    qkT_Q[p][:, SC * s:SC * (s + 1)], psq[si],
                                    qkb_t[:, p:p + 1])
                            else:
                                for h in (0, 1):
                                    nc.vector.tensor_scalar_add(
                                        zqk[p][h][64 * h:64 * h + 64,
                                                  SC * s:SC * (s + 1)],
                                        psq[si][64 * h:64 * h + 64, :],
                                        qkb_t[64 * h:64 * h + 64,
                                              4 + p:5 + p])

            def proj_v():
                """V projection into Vbuf (bf16, [s, col] layout)."""
                for st in range(NKT):
                    psv = qkps.tile([128, 512], F32, name="psv", tag="ps1")
                    for d in range(8):
                        nc.tensor.matmul(
                            psv, xT[d][:, 128 * st:128 * (st + 1)],
                            wv[:, 512 * d:512 * (d + 1)],
                            start=(d == 0), stop=(d == 7))
                    vdst = Vbuf[st].rearrange("p (h c) -> p h c", c=65)[:, :, 0:64]
                    nc.vector.tensor_tensor(
                        vdst, psv.rearrange("p (g c) -> p g c", c=64),
                        bvr_t.rearrange("p (g c) -> p g c", c=64),
                        op=mybir.AluOpType.add)

            def attention(p):
                """Banded attention for pair p over all q chunks."""
                pops = ops_by_p[p]
                op_idx = 0
                for qc in range(NQC):
                    psumO = {}
                    for half in (0, 1):
                        psumO[half] = ops_.tile([65, SC], F32, name="psumO")
                    groups = []
                    while (op_idx < len(pops) and pops[op_idx]["qc"] == qc):
                        o = pops[op_idx]
                        op_idx += 1
                        if groups and groups[-1][0]["kt"] == o["kt"]:
                            groups[-1].append(o)
                        else:
                            groups.append([o])

                    # software pipeline: PV(kt) is emitted after
                    # scores(kt+PIPE) so the PE (strict program order)
                    # never stalls on the exp chain.
                    PIPE = 3
                    pend = []

                    def emit_scores(grp):
                        out = []
                        for o in grp:
                            half, kt = o["half"], o["kt"]
                            c0, c1 = o["c0"], o["c1"]
                            psS = sps.tile([128, SC], F32, name="psS")
                            nc.tensor.matmul(
                                psS[:, c0:c1],
                                zqk[p][half][:, 128 * kt:128 * (kt + 1)],
                                qkT_Q[p][:, SC * qc + c0:SC * qc + c1],
                                start=True, stop=True)
                            eS = expsp.tile([128, SC], BF16, name="eS")
                            for (a, ww, col) in o["exps"]:
                                nc.scalar.activation(
                                    eS[:, a:a + ww], psS[:, a:a + ww], EXP,
                                    bias=expb_t[:, col:col + 1], scale=1.0)
                            if o["tril"]:
                                nc.vector.tensor_mul(
                                    eS[:, c0:c0 + 128],
                                    eS[:, c0:c0 + 128], tril_t)
                            out.append((o, eS))
                        return out

                    def emit_pv(ready):
                        for (o, eS) in ready:
                            c0, c1 = o["c0"], o["c1"]
                            nc.tensor.matmul(
                                psumO[o["half"]][0:65, c0:c1],
                                Vbuf[o["kt"]][:, 65 * o["j"]:65 * o["j"] + 65],
                                eS[:, c0:c1],
                                start=o["first"], stop=o["last"])

                    for grp in groups:
                        pend.append(emit_scores(grp))
                        if len(pend) > PIPE:
                            emit_pv(pend.pop(0))
                    for ready in pend:
                        emit_pv(ready)

                    # normalize both halves (no PE involvement: fast
                    # approx reciprocal + gpsimd partition broadcast).
                    # NOTE: reciprocal_approx_fast cannot read PSUM; the
                    # ssum staging copy is required for correctness.
                    for half in (0, 1):
                        ssum = rcp.tile([1, SC], F32, name="ssum")
                        nc.vector.tensor_copy(ssum, psumO[half][64:65, :])
                        rc = rcp.tile([1, SC], F32, name="rc")
                        nc.vector.reciprocal_approx_fast(rc, ssum)
                        rb = rbp.tile([64, SC], F32, name="rb")
                        nc.gpsimd.partition_broadcast(rb, rc)
                        nc.vector.tensor_mul(
                            OT[p][64 * half:64 * half + 64,
                                  SC * qc:SC * (qc + 1)],
                            psumO[half][0:64, :],
                            rb)

            def out_proj():
                for st in range(NKT):
                    # borrow the attention pools' idle banks so stripe N+1's
                    # matmuls never wait on stripe N's PSUM->SBUF copies
                    pse = [sps.tile([128, SC], F32, name="pse", tag="psS"),
                           ops_.tile([128, SC], F32, name="pse2",
                                     tag="psumO")]
                    for d in range(4):
                        for e in range(2):
                            nc.tensor.matmul(
                                pse[e],
                                OT[d][:, 128 * st:128 * (st + 1)],
                                wo_t[:, 1024 * d + SC * e:
                                     1024 * d + SC * (e + 1)],
                                start=(d == 0), stop=(d == 3))
                    for e in range(2):
                        ob = outp.tile([128, SC], BF16, name="ob")
                        if (st + e) % 2 == 0:
                            nc.vector.tensor_copy(ob, pse[e])
                            nc.gpsimd.dma_start(
                                out=out_d[128 * st:128 * (st + 1),
                                          SC * e:SC * (e + 1)],
                                in_=ob)
                        else:
                            nc.scalar.activation(ob, pse[e], COPY)
                            nc.sync.dma_start(
                                out=out_d[128 * st:128 * (st + 1),
                                          SC * e:SC * (e + 1)],
                                in_=ob)

            # ---- schedule ----
            # PE warm-up on tril (first DMA to land): keeps the PE busy
            # through the input-DMA window so the HAM clock gate reaches
            # 8/8 before the first projection matmul.
            for _ in range(56):
                wps = qkps.tile([128, 128], F32, name="warm", tag="ps1")
                nc.tensor.matmul(wps, tril_t, tril_t, start=True, stop=True)

            ORDER = [3, 2, 1, 0]
            proj_qk(ORDER[0])
            proj_v()
            for i, p in enumerate(ORDER):
                nxt = ORDER[i + 1] if i < 3 else None
                if nxt is not None:
                    wqk_t[nxt] = wqkp.tile([128, 2048], BF16, name="wqk")
                    nc.sync.dma_start(out=wqk_t[nxt], in_=wqk_d[nxt][:, :])
                attention(p)
                if nxt is not None:
                    proj_qk(nxt)
            out_proj()

    nc.compile()
    _nc_cache = nc
    return nc


def make_inputs(x, mask, Wqkv, bqkv, Wo, bo):
    """Build the 8 per-core input maps."""
    import ml_dtypes

    bf16 = ml_dtypes.bfloat16
    x = np.asarray(x, dtype=np.float32)
    Wqkv = np.asarray(Wqkv, dtype=np.float32)
    bqkv = np.asarray(bqkv, dtype=np.float32)
    Wo = np.asarray(Wo, dtype=np.float32)

    # diagonal-block mask in [k_partition, q_column] layout: keep k <= q,
    # i.e. partition p <= column c -> UPPER-triangular
    tril = np.triu(np.ones((128, 128), dtype=bf16))
    vones = np.ones((128, 8), dtype=bf16)
    p_idx = np.arange(128, dtype=np.float32)[:, None]

    in_maps = []
    for c in range(NCORES):
        b, hs = c % 4, c // 4
        heads = [2 * j + hs for j in range(NSLOT)]

        # per-pair packed QK weights: [128, d(8) x (Q128 | K128)]
        # pair p covers slots 2p (partitions 0-63) and 2p+1 (64-127).
        wqk = {}
        for p in range(4):
            h0, h1 = heads[2 * p], heads[2 * p + 1]
            qcols = np.concatenate(
                [np.arange(h0 * HD, h0 * HD + HD),
                 np.arange(h1 * HD, h1 * HD + HD)])
            kcols = D + qcols
            wq = Wqkv[:, qcols] * 0.125            # [D, 128]
            wk = Wqkv[:, kcols]
            pack = np.empty((128, 2048), dtype=np.float32)
            for d in range(8):
                pack[:, 256 * d:256 * d + 128] = wq[128 * d:128 * (d + 1)]
                pack[:, 256 * d + 128:256 * d + 256] = wk[128 * d:128 * (d + 1)]
            wqk[p] = np.ascontiguousarray(pack).astype(bf16)

        # packed V weights [128, d(8) x 512] and Wo [128, f(4) x 1024]
        vcols = np.concatenate(
            [np.arange(2 * D + h * HD, 2 * D + h * HD + HD) for h in heads])
        wvm = Wqkv[:, vcols]                       # [D, 512]
        wvp = np.empty((128, 4096), dtype=np.float32)
        for d in range(8):
            wvp[:, 512 * d:512 * (d + 1)] = wvm[128 * d:128 * (d + 1)]
        rows = np.concatenate(
            [np.arange(h * HD, h * HD + HD) for h in heads])
        wom = Wo[rows, :]                          # [512, 1024]
        wop = np.empty((128, 4096), dtype=np.float32)
        for d in range(4):
            wop[:, 1024 * d:1024 * (d + 1)] = wom[128 * d:128 * (d + 1)]

        # biases: qkb col m -> per-partition bias for (Q pairs 0-3, K pairs
        # 0-3); pair p partitions = slot 2p dims then slot 2p+1 dims
        bq = np.empty((128, 8), dtype=np.float32)
        for p in range(4):
            h0, h1 = heads[2 * p], heads[2 * p + 1]
            qb = np.concatenate([bqkv[h0 * HD:h0 * HD + HD],
                                 bqkv[h1 * HD:h1 * HD + HD]]) * 0.125
            kb = np.concatenate([bqkv[D + h0 * HD:D + h0 * HD + HD],
                                 bqkv[D + h1 * HD:D + h1 * HD + HD]])
            bq[:, p] = qb
            bq[:, 4 + p] = kb
        bvr = np.broadcast_to(bqkv[2 * D:][vcols - 2 * D], (128, 512)).copy()

        expb = np.zeros((128, max(NBIAS, 1)), dtype=np.float32)
        for (j, mkey), col in BIAS_COLS.items():
            expb[:, col:col + 1] = SLOPES[2 * j + hs] * (p_idx - mkey)

        in_maps.append({
            "xT": np.ascontiguousarray(x[b].T).astype(bf16),
            "wqk0": wqk[0], "wqk1": wqk[1], "wqk2": wqk[2], "wqk3": wqk[3],
            "wvp": np.ascontiguousarray(wvp).astype(bf16),
            "wop": np.ascontiguousarray(wop).astype(bf16),
            "qkb": bq,
            "bvr": bvr,
            "expb": expb,
            "tril": tril,
            "vones": vones,
        })
    return in_maps


def kernel(x, mask, Wqkv, bqkv, Wo, bo, _trace=False):
    from concourse.bass_utils import run_bass_kernel_spmd

    nc = build_program()
    in_maps = make_inputs(x, mask, Wqkv, bqkv, Wo, bo)
    res = run_bass_kernel_spmd(nc, in_maps, core_ids=list(range(NCORES)),
                               trace=_trace, trace_cores=[0] if _trace else None)
    bo = np.asarray(bo, dtype=np.float32)
    out = np.empty((B, S, D), dtype=np.float32)
    for b in range(B):
        out[b] = (res.results[b]["out_p"].astype(np.float32)
                  + res.results[b + 4]["out_p"].astype(np.float32) + bo)
    if _trace:
        kernel._last_result = res
    return out
